# revision 1
# baseline (speedup 1.0000x reference)
"""Causal multi-head self-attention on 8 Trainium2 NeuronCores.

Sharding: core = (batch b, head-group g).  B=4 batches x 2 groups of 8 heads
= 8 cores.  Each core computes Q/K/V projections for its 8 heads, causal
attention, and a partial output projection (row-shard of WO); the host sums
the two partials per batch (the tensor-parallel all-reduce, done at gather).

Per-core device pipeline (all matmuls in float32r = full fp32 precision via
the PE's 2-pass mode, 1 cycle/row at N=512):
  stage 1: QT[d',s], KT[d',s] (transposed) and V[s,d'] (natural) projections
           from host-pre-transposed xT and weight shards.
  stage 2: per (head, 512-wide q-block): scoresT[k,q] per 128-wide k-chunk,
           causal mask add on diagonal chunks, exp (no max subtraction --
           scores are O(5) so exp is safe in fp32), attn@V with a ones
           column appended to V so PSUM row 64 accumulates the softmax
           denominator; normalize via reciprocal + K=1 broadcast matmul.
  stage 3: output projection AT^T @ woT -> out rows, DMA to DRAM.
"""

import os
import numpy as np

B, S, D = 4, 2048, 1024
H_TOTAL, DK = 16, 64
G = 2          # head groups (cores per batch)
HG = 8         # heads per core
DG = 512       # head dims per core
CO = 8         # contraction chunks of 128 over D
SBLK = 4       # 512-wide s blocks
QB = 4         # 512-wide q blocks
NEG = -1e9

_BUILD_CACHE = {}


def _build():
    if "nc" in _BUILD_CACHE:
        return _BUILD_CACHE["nc"]

    import concourse.bacc as bacc
    import concourse.mybir as mybir
    import concourse.tile as tile

    f32 = mybir.dt.float32
    f32r = mybir.dt.float32r
    AF = mybir.ActivationFunctionType
    ADD = mybir.AluOpType.add
    MULT = mybir.AluOpType.mult

    nc = bacc.Bacc("TRN2", target_bir_lowering=False)
    xT_d = nc.dram_tensor("xT", [D, S], f32, kind="ExternalInput")
    wq_d = nc.dram_tensor("wqT", [D, DG], f32, kind="ExternalInput")
    wk_d = nc.dram_tensor("wkT", [D, DG], f32, kind="ExternalInput")
    wv_d = nc.dram_tensor("wvT", [D, DG], f32, kind="ExternalInput")
    wo_d = nc.dram_tensor("woT", [DG, D], f32, kind="ExternalInput")
    mask_d = nc.dram_tensor("mask", [128, 4, 512], f32, kind="ExternalInput")
    ones_d = nc.dram_tensor("onesb", [128, 128], f32, kind="ExternalInput")
    out_d = nc.dram_tensor("out", [S, D], f32, kind="ExternalOutput")

    with tile.TileContext(nc) as tc:
        with (
            tc.tile_pool(name="persist", bufs=1) as pp,
            tc.tile_pool(name="psum", bufs=1, space="PSUM") as psp,
        ):
            QT = pp.tile([128, 4, S], f32r, tag="QT")
            KT = pp.tile([128, 4, S], f32r, tag="KT")
            V = pp.tile([128, 16, HG, DK + 1], f32r, tag="V")
            maskb = pp.tile([128, 4, 512], f32, tag="maskb")
            onesb = pp.tile([128, 128], f32, tag="onesb")
            ones_r = pp.tile([1, 64], f32r, tag="ones_r")
            nc.sync.dma_start(maskb[:], mask_d[:, :, :])
            nc.sync.dma_start(onesb[:], ones_d[:, :])
            nc.sync.dma_start(ones_r[:], ones_d[0:1, 0:64].bitcast(f32r))
            # ones column of V (f32 -> f32r rounding copy)
            nc.vector.tensor_copy(
                V[:, :, :, DK : DK + 1],
                onesb[:, 0:128].rearrange("p (so h) -> p so h", so=16)[:, :, :, None],
            )

            # ---------------- stage 1: projections ----------------
            with tc.tile_pool(name="stage1", bufs=1) as s1p:
                wq = s1p.tile([128, CO, DG], f32r, tag="wq")
                wk = s1p.tile([128, CO, DG], f32r, tag="wk")
                wv = s1p.tile([128, CO, DG], f32r, tag="wv")
                nc.sync.dma_start(
                    wq, wq_d[:, :].rearrange("(co ci) d -> ci co d", ci=128).bitcast(f32r)
                )
                nc.sync.dma_start(
                    wk, wk_d[:, :].rearrange("(co ci) d -> ci co d", ci=128).bitcast(f32r)
                )
                nc.sync.dma_start(
                    wv, wv_d[:, :].rearrange("(co ci) d -> ci co d", ci=128).bitcast(f32r)
                )
                for sb in range(SBLK):
                    xt = s1p.tile([128, CO, 512], f32r, tag="xt", bufs=2)
                    nc.sync.dma_start(
                        xt,
                        xT_d[:, sb * 512 : (sb + 1) * 512]
                        .rearrange("(co ci) s -> ci co s", ci=128)
                        .bitcast(f32r),
                    )
                    ssl = slice(sb * 512, (sb + 1) * 512)
                    for do in range(4):
                        dsl = slice(do * 128, (do + 1) * 128)
                        pq = psp.tile([128, 512], f32, tag="proj", bufs=2, name=f"pq{sb}{do}")
                        for co in range(CO):
                            nc.tensor.matmul(
                                pq, wq[:, co, dsl], xt[:, co, :],
                                start=(co == 0), stop=(co == CO - 1),
                            )
                        nc.any.tensor_copy(QT[:, do, ssl], pq[:])
                        pk = psp.tile([128, 512], f32, tag="proj", bufs=2, name=f"pk{sb}{do}")
                        for co in range(CO):
                            nc.tensor.matmul(
                                pk, wk[:, co, dsl], xt[:, co, :],
                                start=(co == 0), stop=(co == CO - 1),
                            )
                        nc.any.tensor_copy(KT[:, do, ssl], pk[:])
                    for so in range(4):
                        pv = psp.tile([128, 512], f32, tag="proj", bufs=2, name=f"pv{sb}{so}")
                        for co in range(CO):
                            nc.tensor.matmul(
                                pv, xt[:, co, so * 128 : (so + 1) * 128], wv[:, co, :],
                                start=(co == 0), stop=(co == CO - 1),
                            )
                        nc.any.tensor_copy(
                            V[:, sb * 4 + so, :, 0:DK],
                            pv[:].rearrange("p (h d) -> p h d", h=HG),
                        )

            # ---------------- stage 2: attention ----------------
            with tc.tile_pool(name="stage2", bufs=1) as s2p:
                AT = s2p.tile([128, 4, S], f32r, tag="AT")
                for pair in range(4):
                    heads = (2 * pair, 2 * pair + 1)
                    for qb in range(QB):
                        qsl = slice(qb * 512, (qb + 1) * 512)
                        ovs = {}
                        for h in heads:
                            ov = psp.tile(
                                [DK + 1, 512], f32, tag="ov", bufs=2, name=f"ov{h}q{qb}"
                            )
                            ovs[h] = ov
                        nkb = 4 * qb + 4
                        for kb in range(nkb):
                            ksl = slice(kb * 128, (kb + 1) * 128)
                            d = kb - 4 * qb
                            for h in heads:
                                base = 64 * (h % 2)
                                psl = slice(base, base + 64)
                                sp = psp.tile(
                                    [128, 512], f32, tag="score", bufs=3,
                                    name=f"sp{h}q{qb}k{kb}",
                                )
                                nc.tensor.matmul(
                                    sp, KT[psl, pair, ksl], QT[psl, pair, qsl],
                                    start=True, stop=True,
                                )
                                if d >= 0:
                                    nc.vector.tensor_tensor(
                                        sp[:], sp[:], maskb[:, d, :], ADD
                                    )
                                et = s2p.tile(
                                    [128, 512], f32r, tag="et", bufs=6,
                                    name=f"et{h}q{qb}k{kb}",
                                )
                                nc.scalar.activation(et[:], sp[:], AF.Exp, scale=0.125)
                                nc.tensor.matmul(
                                    ovs[h], V[:, kb, h, :], et[:],
                                    start=(kb == 0), stop=(kb == nkb - 1),
                                )
                        for h in heads:
                            base = 64 * (h % 2)
                            ov = ovs[h]
                            rs = s2p.tile([1, 512], f32r, tag="rs", bufs=2, name=f"rs{h}{qb}")
                            with nc.allow_low_precision(reason="f32r is fp32-width"):
                                nc.vector.reciprocal(rs[0:1, :], ov[DK : DK + 1, :])
                            rb = psp.tile([64, 512], f32, tag="rb", bufs=1, name=f"rb{h}{qb}")
                            nc.tensor.matmul(
                                rb, ones_r[0:1, :], rs[0:1, :], start=True, stop=True
                            )
                            rbs = s2p.tile([64, 512], f32, tag="rbs", bufs=2, name=f"rbs{h}{qb}")
                            nc.any.tensor_copy(rbs[:], rb[:])
                            nc.vector.tensor_tensor(
                                AT[base : base + 64, pair, qsl], ov[0:DK, :], rbs[:], MULT
                            )

                # ---------------- stage 3: output projection ----------------
                with tc.tile_pool(name="stage3", bufs=1) as s3p:
                    wo = s3p.tile([128, 4, D], f32r, tag="wo")
                    nc.sync.dma_start(
                        wo,
                        wo_d[:, :].rearrange("(io ip) j -> ip io j", ip=128).bitcast(f32r),
                    )
                    for sc in range(16):
                        og = s3p.tile([128, D], f32, tag="og", bufs=2, name=f"og{sc}")
                        for jh in range(2):
                            po = psp.tile(
                                [128, 512], f32, tag="proj", bufs=2, name=f"po{sc}{jh}"
                            )
                            for io in range(4):
                                nc.tensor.matmul(
                                    po,
                                    AT[:, io, sc * 128 : (sc + 1) * 128],
                                    wo[:, io, jh * 512 : (jh + 1) * 512],
                                    start=(io == 0), stop=(io == 3),
                                )
                            nc.any.tensor_copy(og[:, jh * 512 : (jh + 1) * 512], po[:])
                        nc.sync.dma_start(out_d[sc * 128 : (sc + 1) * 128, :], og[:])

    nc.compile()
    _BUILD_CACHE["nc"] = nc
    return nc


def _host_inputs(x, WQ, WK, WV, WO):
    ki = np.arange(128, dtype=np.float32)[:, None, None]
    dd = np.arange(4, dtype=np.float32)[None, :, None]
    qj = np.arange(512, dtype=np.float32)[None, None, :]
    mask = np.where(qj >= ki + 128.0 * dd, 0.0, NEG).astype(np.float32)
    onesb = np.ones((128, 128), dtype=np.float32)

    in_maps = []
    for b in range(B):
        xT = np.ascontiguousarray(x[b].T)
        for g in range(G):
            sl = slice(g * DG, (g + 1) * DG)
            in_maps.append(
                {
                    "xT": xT,
                    "wqT": np.ascontiguousarray(WQ[sl, :].T),
                    "wkT": np.ascontiguousarray(WK[sl, :].T),
                    "wvT": np.ascontiguousarray(WV[sl, :].T),
                    "woT": np.ascontiguousarray(WO[:, sl].T),
                    "mask": mask,
                    "onesb": onesb,
                }
            )
    return in_maps


def kernel(x, WQ, WK, WV, WO):
    from concourse.bass_utils import run_bass_kernel_spmd

    x = np.asarray(x, dtype=np.float32)
    WQ = np.asarray(WQ, dtype=np.float32)
    WK = np.asarray(WK, dtype=np.float32)
    WV = np.asarray(WV, dtype=np.float32)
    WO = np.asarray(WO, dtype=np.float32)

    nc = _build()
    in_maps = _host_inputs(x, WQ, WK, WV, WO)
    res = run_bass_kernel_spmd(
        nc,
        in_maps,
        core_ids=list(range(8)),
        trace=bool(os.environ.get("KERNEL_TRACE")),
    )
    kernel.last_results = res
    parts = [r["out"] for r in res.results]
    out = np.stack([parts[2 * b] + parts[2 * b + 1] for b in range(B)], axis=0)
    return out.astype(np.float32)
